# revision 26
# baseline (speedup 1.0000x reference)
"""Trainium2 Bass kernel for nn_DiffEmbedding1234.

Reference computation (per batch b):
    xt      = x[b].T                                  # [T, C]
    x_diff  = diff(xt) with leading zero row          # [T, C]
    x_emb   = x_diff @ W_ve.T + b_ve                  # [T, D]
    x_sm    = (ewma_fwd(x_emb) + ewma_bwd(x_emb))/2   # [T, D]
    out     = x_sm @ W_lin.T + b_lin                  # [T, D]

Every stage is linear in x, so the whole network collapses to
    out[b] = F @ (x[b].T @ W_comb) + b_out
where
    F      = C_ewma @ D_diff   (T x T, banded: entries decay as 0.9^|lag|)
    W_comb = (W_lin @ W_ve).T  # [C, D]
    b_out  = W_lin @ b_ve + b_lin   (EWMA of a constant is the constant,
                                     so b_ve passes through the smoother)

F's entries decay as 0.9^|lag|, so only near-diagonal blocks matter
(~1e-6 relative truncation, validated end to end vs the reference).

Sharding: data-parallel over batch B=32 -> 8 cores x 4 batches.  The
filter runs along T which stays fully local; small matrices replicated.

Per-core dataflow (all 4 local batches fused into one 128-wide axis
c' = 4*32 channels), all matmuls in fp32r (1 cycle/row vs fp32's 4):
    u^T[c', i-bank] = sum_s (x^T block s).T @ F^T[s-block, bank]   # PE
        - banks of 512 t-outputs, j-window of 5-6 128-blocks,
          full-bank N=512 accumulation in one PSUM bank (x2 ring)
    out[t, e] (per batch, 2-chunk half) = u_b^T.T @ W_comb         # PE
        - stationary u slice at partition base 32*b (row-tiled),
          stride-4 t-columns so out partition p holds t = 4p + kk
        - op PSUM is a 3-deep ring of [128, 1024] so PE never blocks
          on the bias adds
    + bias via DVE add [128, 1024] -> o_sb (3-deep ring)
    -> 1 MiB DMA per (bank, batch) pair, 8 KiB-contiguous per
       partition, alternating between the SP and ACT HWDGE queues;
       one completion semaphore per o_sb slot keeps at most one
       in-flight DMA per semaphore (race-free completion inference).

Measured (interleaved repeat-slope, R=129): ~33-38 us/body/core,
within ~2 us of a pure-DMA replay of the same traffic — HBM-write
bound at ~470-500 GB/s effective per core (16 MiB out + 1 MiB in).

Raw Bass (no Tile): this walrus build allows only ONE sync-wait per
instruction, which Tile's semaphore assignment violates; with explicit
per-engine streams every dependency is a standalone wait_ge and
monotone per-engine counters subsume older deps.

The PJRT executable is cached per program (_get_runner), inputs are
device_put once per call, and outputs are donated on-device zero
buffers — repeated kernel() calls skip re-trace/re-compile/NEFF reload.
"""

import os
import sys

for _p in ("/opt/trn_rl_repo",):
    if os.path.isdir(_p) and _p not in sys.path:
        sys.path.append(_p)

import numpy as np

ALPHA = 0.1
B, C, T, D = 32, 32, 2048, 512
L = 128
NCH = T // L          # 16 chunks of 128 along T
NBK = 4               # banks of 4 chunks (512 t) per batch
NCORES = 8
BPC = B // NCORES     # batches per core
CP = BPC * C          # fused channel axis c' = (b, c) = 128


def _build_filter_banks():
    """F^T slices for the banked scan.

    For output bank m (512 t-values) the contraction runs over j-blocks
    s in [4m-1, 4m+4] (one block of history each side of the bank).
    Returns (fts, bank_terms):
      fts [128, n_uniq*512] with the deduped F^T[s-block, bank-range]
      slices; bank_terms[m] = list of (s, slice_index).
    """
    i = np.arange(T)
    lag = i[:, None] - i[None, :]
    dec = np.where(lag >= 0, 0.9 ** np.clip(lag, 0, None), 0.0)
    A = ALPHA * dec
    A[:, 0] = 0.9 ** i.astype(np.float64)   # x[0] = y[0] boundary
    Bm = A[::-1, ::-1].copy()               # backward EWMA
    Cm = 0.5 * (A + Bm)
    # F = C @ D_diff analytically: D's column j has +1 at row j (j>=1) and
    # -1 at row j+1 (j<=T-2), so F[:, j] = C[:, j]*[j>=1] - C[:, j+1]
    F = np.zeros((T, T))
    F[:, :-1] = -Cm[:, 1:]
    F[:, 1:] += Cm[:, 1:]
    FT = F.T.astype(np.float32)             # FT[j, i]

    uniq: dict[bytes, int] = {}
    slices: list[np.ndarray] = []
    bank_terms: dict[int, list[tuple[int, int]]] = {}
    for m in range(NBK):
        terms = []
        for s in range(4 * m - 1, 4 * m + 5):
            if s < 0 or s >= NCH:
                continue
            blk = FT[s * L:(s + 1) * L, m * 4 * L:(m + 1) * 4 * L]  # [128,512]
            key = blk.tobytes()
            if key not in uniq:
                uniq[key] = len(slices)
                slices.append(blk)
            terms.append((s, uniq[key]))
        bank_terms[m] = terms
    fts = np.concatenate(slices, axis=1)    # [128, n_uniq*512]
    return np.ascontiguousarray(fts, dtype=np.float32), bank_terms


_PROGRAM_CACHE: dict = {}


def _build_program(n_uniq: int, bank_terms, repeats: int = 1):
    """v2: fp32r matmuls (4x PE rate), 2-chunk PSUM op groups with a
    3-deep ring (PE never blocks on the bias adds), output DMAs split
    across the two HWDGE queues (SP + Activation).

    Pipeline per bank m (512 t-rows x 4 local batches):
      PE  : banked scan matmuls (fp32r) -> up_ps[bank%2]
      ACT : copy up_ps -> u_sb[bank%2]
      PE  : per (batch b, half h): 2 proj matmuls -> op_ps[g%3]
      DVE : op_ps[g%3] + bias -> o_sb[q%3] half h    (q = pair = bank*4+b)
      SP/ACT (by b parity): 1 MiB DMA o_sb[q%3] -> y rows of (m, b)
    """
    key = (n_uniq, repeats)
    if key in _PROGRAM_CACHE:
        return _PROGRAM_CACHE[key]

    import concourse.bass as bass
    import concourse.mybir as mybir

    f32 = mybir.dt.float32
    f32r = mybir.dt.float32r
    bf16 = mybir.dt.bfloat16
    ts = bass.ts

    nc = bass.Bass("TRN2")
    xq = nc.dram_tensor("xq", [128, NCH * CP], bf16, kind="ExternalInput")
    fts = nc.dram_tensor("fts", [128, n_uniq * 4 * L], bf16, kind="ExternalInput")
    wcr = nc.dram_tensor("wcr", [CP, D], f32r, kind="ExternalInput")
    bias = nc.dram_tensor("bias", [128, 2 * D], f32, kind="ExternalInput")
    ones = nc.dram_tensor("ones", [128, 128], f32r, kind="ExternalInput")
    biasr = nc.dram_tensor("biasr", [128, D], f32r, kind="ExternalInput")
    y = nc.dram_tensor("y", [BPC, T, D], f32, kind="ExternalOutput")

    xq_sb = [
        nc.alloc_sbuf_tensor(f"xq_sb{i}", [128, NCH * CP], bf16)
        for i in range(2)
    ]
    ft_sb = nc.alloc_sbuf_tensor("ft_sb", [128, n_uniq * 4 * L], bf16)
    wc_sb = nc.alloc_sbuf_tensor("wc_sb", [CP, D], f32r)
    bi_sb = nc.alloc_sbuf_tensor("bi_sb", [128, 2 * D], f32)
    on_sb = nc.alloc_sbuf_tensor("on_sb", [128, 128], f32r)
    br_sb = nc.alloc_sbuf_tensor("br_sb", [128, D], f32r)
    u_sb = [nc.alloc_sbuf_tensor(f"u{i}", [128, 4 * L], f32r) for i in range(2)]
    o_sb = [nc.alloc_sbuf_tensor(f"o{i}", [128, 4 * D], f32) for i in range(3)]
    up_ps = [nc.alloc_psum_tensor(f"up{i}", [128, 4 * L], f32) for i in range(2)]
    op_ps = [nc.alloc_psum_tensor(f"op{i}", [128, 2 * D], f32) for i in range(3)]

    R = repeats
    NPAIR = NBK * BPC          # 16 output pairs (bank, batch) per repeat
    NB = R * NBK               # total banks

    # replay PE counter matching the SOFTWARE-PIPELINED PE order:
    #   scan(0); for k: [scan(k+1)]; proj(k)
    # so the ACT copy of bank k overlaps scan(k+1) on the PE instead of
    # stalling the projection.
    scan_done = {}             # bank -> s_pe value
    op_done = {}               # g -> s_pe value
    pe = 1
    scan_done[0] = 1
    for k in range(NB):
        if k + 1 < NB:
            pe += 1
            scan_done[k + 1] = pe
        for b in range(BPC):
            for h in range(2):
                pe += 1
                op_done[2 * (k * BPC + b) + h] = pe

    # output DMA bookkeeping: pair q's DMA increments slot sem q%3 (the
    # q -> q+3 o_sb slot chain guarantees at most one in-flight DMA per
    # sem, so completion-order inference is race-free).  Queue (SP vs
    # ACT) alternates with batch parity, independent of the slot sem.
    slot_total = [0, 0, 0]
    pair_dma = {}
    for q in range(R * NPAIR):
        s = q % 3
        slot_total[s] += 1
        pair_dma[q] = (s, 16 * slot_total[s])

    # PSUM->SBUF move ownership: DVE does tensor_add (bias fused) for 5 of
    # every 8 pairs; ACT does plain copies for the other 3, whose bias is
    # injected into PSUM by K=1 PE matmuls (ones x biasr outer product).
    ACT_OWN = frozenset({5, 6, 7})

    def owner_of(q):
        return "mv" if q % 8 in ACT_OWN else "dve"

    half_move = {}             # g -> (owner, count after that half's move)
    move_done = {}             # q -> (owner, count after both halves moved)
    _dve_n = _mv_n = 0
    for q in range(R * NPAIR):
        ow = owner_of(q)
        for h in range(2):
            if ow == "dve":
                _dve_n += 1
                half_move[2 * q + h] = (ow, _dve_n)
            else:
                _mv_n += 1
                half_move[2 * q + h] = (ow, _mv_n)
        move_done[q] = half_move[2 * q + 1]

    def y_view(q):
        # partition p carries rows t = 4p + kk (kk = 0..3), so each
        # partition's [kk, e] block is one contiguous 8 KiB DRAM span
        # (4 consecutive 2 KiB rows) -> 128 big DMA descriptors instead
        # of 512 small ones.
        bank = q // BPC
        m = bank % NBK
        b = q % BPC
        return y[b, m * 4 * L:(m + 1) * 4 * L, :].rearrange(
            "(p kk) e -> p kk e", kk=4
        )

    def o_view(q):
        return o_sb[q % 3][:].rearrange("p (kk e) -> p kk e", e=D)

    with (
        nc.semaphore("s_const") as s_const,
        nc.semaphore("s_x0") as s_x0,
        nc.semaphore("s_x1") as s_x1,
        nc.semaphore("s_sl0") as s_sl0,
        nc.semaphore("s_sl1") as s_sl1,
        nc.semaphore("s_sl2") as s_sl2,
        nc.semaphore("s_pe") as s_pe,
        nc.semaphore("s_act") as s_act,
        nc.semaphore("s_dve") as s_dve,
        nc.semaphore("s_mv") as s_mv,
    ):
        s_slot = [s_sl0, s_sl1, s_sl2]
        s_x = [s_x0, s_x1]
        s_move = {"dve": s_dve, "mv": s_mv}

        def scan_mms(k):
            # scan of global bank k into up_ps[k%2]; reads xq slot of its
            # repeat.  up_ps slot release (copy(k-2) done) is subsumed by
            # proj(k-2)'s s_act wait, which precedes this in PE order.
            r, m = divmod(k, NBK)
            terms = bank_terms[m]
            up = up_ps[k % 2]
            xs = xq_sb[r % 2]
            if m == 0:
                nc.tensor.wait_ge(s_x[r % 2], 16 * (r // 2 + 1))
            for n, (s, sl) in enumerate(terms):
                mm = nc.tensor.matmul(
                    up[:],
                    xs[:, ts(s, CP)],
                    ft_sb[:, ts(sl, 4 * L)],
                    start=(n == 0),
                    stop=(n == len(terms) - 1),
                )
            mm.then_inc(s_pe, 1)

        with nc.Block() as block:

            @block.sync
            def _(sync):
                sync.dma_start(ft_sb[:], fts[:]).then_inc(s_const, 16)
                sync.dma_start(wc_sb[:], wcr[:]).then_inc(s_const, 16)
                sync.dma_start(bi_sb[:], bias[:]).then_inc(s_const, 16)
                sync.dma_start(on_sb[:], ones[:]).then_inc(s_const, 16)
                sync.dma_start(br_sb[:], biasr[:]).then_inc(s_const, 16)
                sync.dma_start(xq_sb[0][:], xq[:]).then_inc(s_x[0], 16)
                for r in range(R):
                    if r + 1 < R:
                        # prefetch next repeat's xq into slot (r+1)%2;
                        # slot free once repeat r-1's scans are done
                        if r >= 1:
                            sync.wait_ge(s_pe, scan_done[r * NBK - 1])
                        sync.dma_start(
                            xq_sb[(r + 1) % 2][:], xq[:]
                        ).then_inc(s_x[(r + 1) % 2], 16)
                    for q in range(r * NPAIR, (r + 1) * NPAIR):
                        if (q % BPC) % 2 != 0:
                            continue
                        ow, cnt = move_done[q]
                        sync.wait_ge(s_move[ow], cnt)  # both halves moved
                        sync.dma_start(y_view(q), o_view(q)).then_inc(
                            s_slot[q % 3], 16
                        )
                # drain: all output DMAs landed
                for s in range(3):
                    sync.wait_ge(s_slot[s], 16 * slot_total[s])

            @block.tensor
            def _(tensor):
                tensor.wait_ge(s_const, 80)
                scan_mms(0)
                for k in range(NB):
                    if k + 1 < NB:
                        scan_mms(k + 1)
                    # proj for bank k needs its u copy (also releases
                    # up_ps[k%2] for scan(k+2))
                    tensor.wait_ge(s_act, k + 1)
                    u = u_sb[k % 2]
                    for b in range(BPC):
                        ow = owner_of(k * BPC + b)
                        for h in range(2):
                            g = 2 * (k * BPC + b) + h
                            if g >= 3:
                                # op_ps[g%3] free once move(g-3) done
                                mo, cnt = half_move[g - 3]
                                tensor.wait_ge(s_move[mo], cnt)
                            for j in range(2):
                                kk = 2 * h + j
                                # stride-4 t-columns: out partition p
                                # holds t = 4p + kk, making the y DMA
                                # 8 KiB-contiguous per partition
                                mm = nc.tensor.matmul(
                                    op_ps[g % 3][:, ts(j, D)],
                                    u[b * C:(b + 1) * C, kk:4 * L:4],
                                    wc_sb[b * C:(b + 1) * C, :],
                                    start=True, stop=(ow == "dve"),
                                    tile_position=(b * C, 0),
                                )
                                if ow == "mv":
                                    # bias into PSUM so ACT can move with a
                                    # plain copy: += ones.T @ biasr.  Same
                                    # tile as the proj mm so the PSUM
                                    # accumulation group opens and closes
                                    # on one tile.
                                    mm = nc.tensor.matmul(
                                        op_ps[g % 3][:, ts(j, D)],
                                        on_sb[b * C:(b + 1) * C, :],
                                        br_sb[b * C:(b + 1) * C, :],
                                        start=False, stop=True,
                                        tile_position=(b * C, 0),
                                    )
                            mm.then_inc(s_pe, 1)

            @block.scalar
            def _(scalar):
                # copy(k+1) is emitted BEFORE bank k's output DMAs so the
                # next bank's u is never stuck behind DMA issue.
                scalar.wait_ge(s_pe, scan_done[0])
                nc.scalar.copy(
                    u_sb[0][:], up_ps[0][:]
                ).then_inc(s_act, 1)
                for k in range(NB):
                    if k + 1 < NB:
                        # u_sb[(k+1)%2] overwrite safe: its reader proj(k-1)
                        # precedes scan(k+1) in PE order
                        scalar.wait_ge(s_pe, scan_done[k + 1])
                        nc.scalar.copy(
                            u_sb[(k + 1) % 2][:], up_ps[(k + 1) % 2][:]
                        ).then_inc(s_act, 1)
                    for b in range(BPC):
                        q = k * BPC + b
                        if owner_of(q) == "mv":
                            for h in range(2):
                                g = 2 * q + h
                                if h == 0 and q >= 3:
                                    s, cnt = pair_dma[q - 3]
                                    scalar.wait_ge(s_slot[s], cnt)
                                scalar.wait_ge(s_pe, op_done[g])
                                nc.scalar.copy(
                                    o_sb[q % 3][:, ts(h, 2 * D)],
                                    op_ps[g % 3][:],
                                ).then_inc(s_mv, 1)
                        if b % 2 == 1:
                            # DMA data is read by the async SDMA engine, so
                            # queue FIFO order does NOT order it after the
                            # copies — always wait on the move semaphore.
                            ow2, cnt2 = move_done[q]
                            scalar.wait_ge(s_move[ow2], cnt2)
                            scalar.dma_start(
                                y_view(q), o_view(q)
                            ).then_inc(s_slot[q % 3], 16)

            @block.vector
            def _(vector):
                vector.wait_ge(s_const, 80)
                for q in range(R * NPAIR):
                    if owner_of(q) != "dve":
                        continue
                    for h in range(2):
                        g = 2 * q + h
                        if h == 0 and q >= 3:
                            # o_sb[q%3] free once DMA(q-3) landed
                            s, cnt = pair_dma[q - 3]
                            vector.wait_ge(s_slot[s], cnt)
                        vector.wait_ge(s_pe, op_done[g])
                        nc.vector.tensor_add(
                            o_sb[q % 3][:, ts(h, 2 * D)],
                            op_ps[g % 3][:],
                            bi_sb[:],
                        ).then_inc(s_dve, 1)

    _PROGRAM_CACHE[key] = nc
    return nc


def _prep_inputs(x, W_ve, b_ve, W_lin, b_lin):
    fts, bank_terms = _build_filter_banks()
    n_uniq = fts.shape[1] // (4 * L)
    W_comb = (W_lin.astype(np.float64) @ W_ve.astype(np.float64)).T  # [C, D]
    b_out = W_lin.astype(np.float64) @ b_ve.astype(np.float64) + b_lin.astype(np.float64)
    # xq[p, k*CP + b*C + c] = x[b, c, k*128 + p]
    xq_all = (
        x.reshape(B, C, NCH, L)
        .transpose(3, 2, 0, 1)           # [p, k, b, c]  (b within full B)
        .reshape(L, NCH, B, C)
    )
    wcr = np.tile(W_comb.astype(np.float32), (BPC, 1))          # [128, D]
    bias2 = np.tile(b_out.astype(np.float32), 2)                 # [2*D]
    import ml_dtypes
    bf16 = ml_dtypes.bfloat16
    common = {
        "fts": fts.astype(bf16),
        "wcr": np.ascontiguousarray(wcr),
        "bias": np.ascontiguousarray(
            np.broadcast_to(bias2.astype(np.float32), (128, 2 * D))
        ),
        # row 0 of each 32-partition batch strip is ones, rest zeros, so
        # the K=32 bias matmul adds exactly one copy of b_out
        "ones": np.ascontiguousarray(
            (np.arange(128)[:, None] % C == 0).astype(np.float32)
            * np.ones((1, 128), np.float32)
        ),
        "biasr": np.ascontiguousarray(
            np.broadcast_to(b_out.astype(np.float32), (128, D))
        ),
    }
    in_maps = []
    for cc in range(NCORES):
        xq = xq_all[:, :, cc * BPC:(cc + 1) * BPC, :].reshape(L, NCH * CP)
        in_maps.append(
            {"xq": np.ascontiguousarray(xq).astype(bf16), **common}
        )
    return in_maps, n_uniq, bank_terms


_RUNNER_CACHE: dict = {}


def _get_runner(n_uniq: int, bank_terms, repeats: int = 1):
    """Compile the Bass program once per `repeats` and cache the jitted
    PJRT executable so repeated kernel invocations skip re-trace /
    re-compile / NEFF re-load."""
    key = (n_uniq, repeats)
    if key in _RUNNER_CACHE:
        return _RUNNER_CACHE[key]

    import jax
    from jax.experimental.shard_map import shard_map
    from jax.sharding import Mesh, NamedSharding, PartitionSpec

    from concourse import bass2jax, mybir

    bass2jax.install_neuronx_cc_hook()
    nc = _build_program(n_uniq, bank_terms, repeats=repeats)

    partition_name = (
        nc.partition_id_tensor.name if nc.partition_id_tensor else None
    )
    in_names: list[str] = []
    out_names: list[str] = []
    out_avals = []
    for alloc in nc.m.functions[0].allocations:
        if not isinstance(alloc, mybir.MemoryLocationSet):
            continue
        name = alloc.memorylocations[0].name
        if alloc.kind == "ExternalInput":
            if name != partition_name:
                in_names.append(name)
        elif alloc.kind == "ExternalOutput":
            out_names.append(name)
            out_avals.append(
                jax.core.ShapedArray(
                    tuple(alloc.tensor_shape), mybir.dt.np(alloc.dtype)
                )
            )
    n_params = len(in_names)
    n_outs = len(out_names)
    all_in = tuple(in_names) + tuple(out_names)
    if partition_name is not None:
        all_in = all_in + (partition_name,)

    def _body(*args):
        operands = list(args)
        if partition_name is not None:
            operands.append(bass2jax.partition_id_tensor())
        return tuple(
            bass2jax._bass_exec_p.bind(
                *operands,
                out_avals=tuple(out_avals),
                in_names=all_in,
                out_names=tuple(out_names),
                lowering_input_output_aliases=(),
                sim_require_finite=True,
                sim_require_nnan=True,
                nc=nc,
            )
        )

    devices = jax.devices()[:NCORES]
    mesh = Mesh(np.asarray(devices), ("core",))
    sharding = NamedSharding(mesh, PartitionSpec("core"))
    sharded = jax.jit(
        shard_map(
            _body,
            mesh=mesh,
            in_specs=(PartitionSpec("core"),) * (n_params + n_outs),
            out_specs=(PartitionSpec("core"),) * n_outs,
            check_rep=False,
        ),
        donate_argnums=tuple(range(n_params, n_params + n_outs)),
        keep_unused=True,
    )

    import jax.numpy as jnp

    out_shapes = [
        (NCORES * a.shape[0], *a.shape[1:]) for a in out_avals
    ]
    out_dtypes = [a.dtype for a in out_avals]

    zeros_fn = jax.jit(
        lambda: tuple(
            jnp.zeros(s, d) for s, d in zip(out_shapes, out_dtypes)
        ),
        out_shardings=(sharding,) * n_outs,
    )

    runner = {
        "nc": nc,
        "fn": sharded,
        "zeros_fn": zeros_fn,
        "in_names": in_names,
        "out_names": out_names,
        "sharding": sharding,
        "out_shapes": out_shapes,
    }
    _RUNNER_CACHE[key] = runner
    return runner


def _device_inputs(runner, in_maps):
    """Concat per-core input maps along axis 0 and put on the mesh."""
    import jax

    arrs = []
    for name in runner["in_names"]:
        cat = np.concatenate([m[name] for m in in_maps], axis=0)
        arrs.append(jax.device_put(cat, runner["sharding"]))
    return arrs


def _run(in_maps, n_uniq, bank_terms, repeats: int = 1, fetch: bool = True,
         dev_inputs=None):
    import jax

    runner = _get_runner(n_uniq, bank_terms, repeats=repeats)
    if dev_inputs is None:
        dev_inputs = _device_inputs(runner, in_maps)
    zeros = runner["zeros_fn"]()
    outs = runner["fn"](*dev_inputs, *zeros)
    if fetch:
        return {n: np.asarray(o) for n, o in zip(runner["out_names"], outs)}
    jax.block_until_ready(outs)
    return None


def kernel(x, W_ve, b_ve, W_lin, b_lin):
    in_maps, n_uniq, bank_terms = _prep_inputs(x, W_ve, b_ve, W_lin, b_lin)
    res = _run(in_maps, n_uniq, bank_terms)
    # cores stacked along axis 0: core c holds batches [4c, 4c+4)
    return np.ascontiguousarray(res["y"]).astype(np.float32, copy=False)



# revision 27
# speedup vs baseline: 1.0868x; 1.0868x over previous
"""Trainium2 Bass kernel for nn_DiffEmbedding1234.

Reference computation (per batch b):
    xt      = x[b].T                                  # [T, C]
    x_diff  = diff(xt) with leading zero row          # [T, C]
    x_emb   = x_diff @ W_ve.T + b_ve                  # [T, D]
    x_sm    = (ewma_fwd(x_emb) + ewma_bwd(x_emb))/2   # [T, D]
    out     = x_sm @ W_lin.T + b_lin                  # [T, D]

Every stage is linear in x, so the whole network collapses to
    out[b] = F @ (x[b].T @ W_comb) + b_out
where
    F      = C_ewma @ D_diff   (T x T, banded: entries decay as 0.9^|lag|)
    W_comb = (W_lin @ W_ve).T  # [C, D]
    b_out  = W_lin @ b_ve + b_lin   (EWMA of a constant is the constant,
                                     so b_ve passes through the smoother)

F's entries decay as 0.9^|lag|, so only near-diagonal blocks matter
(~1e-6 relative truncation, validated end to end vs the reference).

Sharding: data-parallel over batch B=32 -> 8 cores x 4 batches.  The
filter runs along T which stays fully local; small matrices replicated.

Per-core dataflow (all 4 local batches fused into one 128-wide axis
c' = 4*32 channels), all matmuls in fp32r (1 cycle/row vs fp32's 4):
    u^T[c', i-bank] = sum_s (x^T block s).T @ F^T[s-block, bank]   # PE
        - banks of 512 t-outputs, j-window of 5-6 128-blocks,
          full-bank N=512 accumulation in one PSUM bank (x2 ring)
    out[t, e] (per batch, 2-chunk half) = u_b^T.T @ W_comb         # PE
        - stationary u slice at partition base 32*b (row-tiled),
          stride-4 t-columns so out partition p holds t = 4p + kk
        - op PSUM is a 3-deep ring of [128, 1024] so PE never blocks
          on the bias adds
    + bias via DVE add [128, 1024] -> o_sb (3-deep ring)
    -> 1 MiB DMA per (bank, batch) pair, 8 KiB-contiguous per
       partition, alternating between the SP and ACT HWDGE queues;
       one completion semaphore per o_sb slot keeps at most one
       in-flight DMA per semaphore (race-free completion inference).

Measured (interleaved repeat-slope, R=129): ~33-38 us/body/core,
within ~2 us of a pure-DMA replay of the same traffic — HBM-write
bound at ~470-500 GB/s effective per core (16 MiB out + 1 MiB in).

Raw Bass (no Tile): this walrus build allows only ONE sync-wait per
instruction, which Tile's semaphore assignment violates; with explicit
per-engine streams every dependency is a standalone wait_ge and
monotone per-engine counters subsume older deps.

The PJRT executable is cached per program (_get_runner), inputs are
device_put once per call, and outputs are donated on-device zero
buffers — repeated kernel() calls skip re-trace/re-compile/NEFF reload.
"""

import os
import sys

for _p in ("/opt/trn_rl_repo",):
    if os.path.isdir(_p) and _p not in sys.path:
        sys.path.append(_p)

import numpy as np

ALPHA = 0.1
B, C, T, D = 32, 32, 2048, 512
L = 128
NCH = T // L          # 16 chunks of 128 along T
NBK = 4               # banks of 4 chunks (512 t) per batch
NCORES = 8
BPC = B // NCORES     # batches per core
CP = BPC * C          # fused channel axis c' = (b, c) = 128


def _build_filter_banks():
    """F^T slices for the banked scan.

    For output bank m (512 t-values) the contraction runs over j-blocks
    s in [4m-1, 4m+4] (one block of history each side of the bank).
    Returns (fts, bank_terms):
      fts [128, n_uniq*512] with the deduped F^T[s-block, bank-range]
      slices; bank_terms[m] = list of (s, slice_index).
    """
    i = np.arange(T)
    lag = i[:, None] - i[None, :]
    dec = np.where(lag >= 0, 0.9 ** np.clip(lag, 0, None), 0.0)
    A = ALPHA * dec
    A[:, 0] = 0.9 ** i.astype(np.float64)   # x[0] = y[0] boundary
    Bm = A[::-1, ::-1].copy()               # backward EWMA
    Cm = 0.5 * (A + Bm)
    # F = C @ D_diff analytically: D's column j has +1 at row j (j>=1) and
    # -1 at row j+1 (j<=T-2), so F[:, j] = C[:, j]*[j>=1] - C[:, j+1]
    F = np.zeros((T, T))
    F[:, :-1] = -Cm[:, 1:]
    F[:, 1:] += Cm[:, 1:]
    FT = F.T.astype(np.float32)             # FT[j, i]

    uniq: dict[bytes, int] = {}
    slices: list[np.ndarray] = []
    bank_terms: dict[int, list[tuple[int, int]]] = {}
    for m in range(NBK):
        terms = []
        for s in range(4 * m - 1, 4 * m + 5):
            if s < 0 or s >= NCH:
                continue
            blk = FT[s * L:(s + 1) * L, m * 4 * L:(m + 1) * 4 * L]  # [128,512]
            key = blk.tobytes()
            if key not in uniq:
                uniq[key] = len(slices)
                slices.append(blk)
            terms.append((s, uniq[key]))
        bank_terms[m] = terms
    fts = np.concatenate(slices, axis=1)    # [128, n_uniq*512]
    return np.ascontiguousarray(fts, dtype=np.float32), bank_terms


_PROGRAM_CACHE: dict = {}


def _build_program(n_uniq: int, bank_terms, repeats: int = 1):
    """v2: fp32r matmuls (4x PE rate), 2-chunk PSUM op groups with a
    3-deep ring (PE never blocks on the bias adds), output DMAs split
    across the two HWDGE queues (SP + Activation).

    Pipeline per bank m (512 t-rows x 4 local batches):
      PE  : banked scan matmuls (fp32r) -> up_ps[bank%2]
      ACT : copy up_ps -> u_sb[bank%2]
      PE  : per (batch b, half h): 2 proj matmuls -> op_ps[g%3]
      DVE : op_ps[g%3] + bias -> o_sb[q%3] half h    (q = pair = bank*4+b)
      SP/ACT (by b parity): 1 MiB DMA o_sb[q%3] -> y rows of (m, b)
    """
    key = (n_uniq, repeats)
    if key in _PROGRAM_CACHE:
        return _PROGRAM_CACHE[key]

    import concourse.bass as bass
    import concourse.mybir as mybir

    f32 = mybir.dt.float32
    f32r = mybir.dt.float32r
    ts = bass.ts

    nc = bass.Bass("TRN2")
    xq = nc.dram_tensor("xq", [128, NCH * CP], f32r, kind="ExternalInput")
    fts = nc.dram_tensor("fts", [128, n_uniq * 4 * L], f32r, kind="ExternalInput")
    wcr = nc.dram_tensor("wcr", [CP, D], f32r, kind="ExternalInput")
    bias = nc.dram_tensor("bias", [128, 2 * D], f32, kind="ExternalInput")
    ones = nc.dram_tensor("ones", [128, 128], f32r, kind="ExternalInput")
    biasr = nc.dram_tensor("biasr", [128, D], f32r, kind="ExternalInput")
    y = nc.dram_tensor("y", [BPC, T, D], f32, kind="ExternalOutput")

    xq_sb = [
        nc.alloc_sbuf_tensor(f"xq_sb{i}", [128, NCH * CP], f32r)
        for i in range(2)
    ]
    ft_sb = nc.alloc_sbuf_tensor("ft_sb", [128, n_uniq * 4 * L], f32r)
    wc_sb = nc.alloc_sbuf_tensor("wc_sb", [CP, D], f32r)
    bi_sb = nc.alloc_sbuf_tensor("bi_sb", [128, 2 * D], f32)
    on_sb = nc.alloc_sbuf_tensor("on_sb", [128, 128], f32r)
    br_sb = nc.alloc_sbuf_tensor("br_sb", [128, D], f32r)
    u_sb = [nc.alloc_sbuf_tensor(f"u{i}", [128, 4 * L], f32r) for i in range(2)]
    o_sb = [nc.alloc_sbuf_tensor(f"o{i}", [128, 4 * D], f32) for i in range(3)]
    up_ps = [nc.alloc_psum_tensor(f"up{i}", [128, 4 * L], f32) for i in range(2)]
    op_ps = [nc.alloc_psum_tensor(f"op{i}", [128, 2 * D], f32) for i in range(3)]

    R = repeats
    NPAIR = NBK * BPC          # 16 output pairs (bank, batch) per repeat
    NB = R * NBK               # total banks

    # replay PE counter matching the SOFTWARE-PIPELINED PE order:
    #   scan(0); for k: [scan(k+1)]; proj(k)
    # so the ACT copy of bank k overlaps scan(k+1) on the PE instead of
    # stalling the projection.
    scan_done = {}             # bank -> s_pe value
    op_done = {}               # g -> s_pe value
    pe = 1
    scan_done[0] = 1
    for k in range(NB):
        if k + 1 < NB:
            pe += 1
            scan_done[k + 1] = pe
        for b in range(BPC):
            for h in range(2):
                pe += 1
                op_done[2 * (k * BPC + b) + h] = pe

    # output DMA bookkeeping: pair q's DMA increments slot sem q%3 (the
    # q -> q+3 o_sb slot chain guarantees at most one in-flight DMA per
    # sem, so completion-order inference is race-free).  Queue (SP vs
    # ACT) alternates with batch parity, independent of the slot sem.
    slot_total = [0, 0, 0]
    pair_dma = {}
    for q in range(R * NPAIR):
        s = q % 3
        slot_total[s] += 1
        pair_dma[q] = (s, 16 * slot_total[s])

    # PSUM->SBUF move ownership: DVE does tensor_add (bias fused) for 5 of
    # every 8 pairs; ACT does plain copies for the other 3, whose bias is
    # injected into PSUM by K=1 PE matmuls (ones x biasr outer product).
    ACT_OWN = frozenset({5, 6, 7})

    def owner_of(q):
        return "mv" if q % 8 in ACT_OWN else "dve"

    half_move = {}             # g -> (owner, count after that half's move)
    move_done = {}             # q -> (owner, count after both halves moved)
    _dve_n = _mv_n = 0
    for q in range(R * NPAIR):
        ow = owner_of(q)
        for h in range(2):
            if ow == "dve":
                _dve_n += 1
                half_move[2 * q + h] = (ow, _dve_n)
            else:
                _mv_n += 1
                half_move[2 * q + h] = (ow, _mv_n)
        move_done[q] = half_move[2 * q + 1]

    def y_view(q):
        # partition p carries rows t = 4p + kk (kk = 0..3), so each
        # partition's [kk, e] block is one contiguous 8 KiB DRAM span
        # (4 consecutive 2 KiB rows) -> 128 big DMA descriptors instead
        # of 512 small ones.
        bank = q // BPC
        m = bank % NBK
        b = q % BPC
        return y[b, m * 4 * L:(m + 1) * 4 * L, :].rearrange(
            "(p kk) e -> p kk e", kk=4
        )

    def o_view(q):
        return o_sb[q % 3][:].rearrange("p (kk e) -> p kk e", e=D)

    with (
        nc.semaphore("s_const") as s_const,
        nc.semaphore("s_x0") as s_x0,
        nc.semaphore("s_x1") as s_x1,
        nc.semaphore("s_sl0") as s_sl0,
        nc.semaphore("s_sl1") as s_sl1,
        nc.semaphore("s_sl2") as s_sl2,
        nc.semaphore("s_pe") as s_pe,
        nc.semaphore("s_act") as s_act,
        nc.semaphore("s_dve") as s_dve,
        nc.semaphore("s_mv") as s_mv,
    ):
        s_slot = [s_sl0, s_sl1, s_sl2]
        s_x = [s_x0, s_x1]
        s_move = {"dve": s_dve, "mv": s_mv}

        def scan_mms(k):
            # scan of global bank k into up_ps[k%2]; reads xq slot of its
            # repeat.  up_ps slot release (copy(k-2) done) is subsumed by
            # proj(k-2)'s s_act wait, which precedes this in PE order.
            r, m = divmod(k, NBK)
            terms = bank_terms[m]
            up = up_ps[k % 2]
            xs = xq_sb[r % 2]
            if m == 0:
                nc.tensor.wait_ge(s_x[r % 2], 16 * (r // 2 + 1))
            for n, (s, sl) in enumerate(terms):
                mm = nc.tensor.matmul(
                    up[:],
                    xs[:, ts(s, CP)],
                    ft_sb[:, ts(sl, 4 * L)],
                    start=(n == 0),
                    stop=(n == len(terms) - 1),
                )
            mm.then_inc(s_pe, 1)

        with nc.Block() as block:

            @block.sync
            def _(sync):
                sync.dma_start(ft_sb[:], fts[:]).then_inc(s_const, 16)
                sync.dma_start(wc_sb[:], wcr[:]).then_inc(s_const, 16)
                sync.dma_start(bi_sb[:], bias[:]).then_inc(s_const, 16)
                sync.dma_start(on_sb[:], ones[:]).then_inc(s_const, 16)
                sync.dma_start(br_sb[:], biasr[:]).then_inc(s_const, 16)
                sync.dma_start(xq_sb[0][:], xq[:]).then_inc(s_x[0], 16)
                for r in range(R):
                    if r + 1 < R:
                        # prefetch next repeat's xq into slot (r+1)%2;
                        # slot free once repeat r-1's scans are done
                        if r >= 1:
                            sync.wait_ge(s_pe, scan_done[r * NBK - 1])
                        sync.dma_start(
                            xq_sb[(r + 1) % 2][:], xq[:]
                        ).then_inc(s_x[(r + 1) % 2], 16)
                    for q in range(r * NPAIR, (r + 1) * NPAIR):
                        if (q % BPC) % 2 != 0:
                            continue
                        ow, cnt = move_done[q]
                        sync.wait_ge(s_move[ow], cnt)  # both halves moved
                        sync.dma_start(y_view(q), o_view(q)).then_inc(
                            s_slot[q % 3], 16
                        )
                # drain: all output DMAs landed
                for s in range(3):
                    sync.wait_ge(s_slot[s], 16 * slot_total[s])

            @block.tensor
            def _(tensor):
                tensor.wait_ge(s_const, 80)
                scan_mms(0)
                for k in range(NB):
                    if k + 1 < NB:
                        scan_mms(k + 1)
                    # proj for bank k needs its u copy (also releases
                    # up_ps[k%2] for scan(k+2))
                    tensor.wait_ge(s_act, k + 1)
                    u = u_sb[k % 2]
                    for b in range(BPC):
                        ow = owner_of(k * BPC + b)
                        for h in range(2):
                            g = 2 * (k * BPC + b) + h
                            if g >= 3:
                                # op_ps[g%3] free once move(g-3) done
                                mo, cnt = half_move[g - 3]
                                tensor.wait_ge(s_move[mo], cnt)
                            for j in range(2):
                                kk = 2 * h + j
                                # stride-4 t-columns: out partition p
                                # holds t = 4p + kk, making the y DMA
                                # 8 KiB-contiguous per partition
                                mm = nc.tensor.matmul(
                                    op_ps[g % 3][:, ts(j, D)],
                                    u[b * C:(b + 1) * C, kk:4 * L:4],
                                    wc_sb[b * C:(b + 1) * C, :],
                                    start=True, stop=(ow == "dve"),
                                    tile_position=(b * C, 0),
                                )
                                if ow == "mv":
                                    # bias into PSUM so ACT can move with a
                                    # plain copy: += ones.T @ biasr.  Same
                                    # tile as the proj mm so the PSUM
                                    # accumulation group opens and closes
                                    # on one tile.
                                    mm = nc.tensor.matmul(
                                        op_ps[g % 3][:, ts(j, D)],
                                        on_sb[b * C:(b + 1) * C, :],
                                        br_sb[b * C:(b + 1) * C, :],
                                        start=False, stop=True,
                                        tile_position=(b * C, 0),
                                    )
                            mm.then_inc(s_pe, 1)

            @block.scalar
            def _(scalar):
                # copy(k+1) is emitted BEFORE bank k's output DMAs so the
                # next bank's u is never stuck behind DMA issue.
                scalar.wait_ge(s_pe, scan_done[0])
                nc.scalar.copy(
                    u_sb[0][:], up_ps[0][:]
                ).then_inc(s_act, 1)
                for k in range(NB):
                    if k + 1 < NB:
                        # u_sb[(k+1)%2] overwrite safe: its reader proj(k-1)
                        # precedes scan(k+1) in PE order
                        scalar.wait_ge(s_pe, scan_done[k + 1])
                        nc.scalar.copy(
                            u_sb[(k + 1) % 2][:], up_ps[(k + 1) % 2][:]
                        ).then_inc(s_act, 1)
                    for b in range(BPC):
                        q = k * BPC + b
                        if owner_of(q) == "mv":
                            for h in range(2):
                                g = 2 * q + h
                                if h == 0 and q >= 3:
                                    s, cnt = pair_dma[q - 3]
                                    scalar.wait_ge(s_slot[s], cnt)
                                scalar.wait_ge(s_pe, op_done[g])
                                nc.scalar.copy(
                                    o_sb[q % 3][:, ts(h, 2 * D)],
                                    op_ps[g % 3][:],
                                ).then_inc(s_mv, 1)
                        if b % 2 == 1:
                            # DMA data is read by the async SDMA engine, so
                            # queue FIFO order does NOT order it after the
                            # copies — always wait on the move semaphore.
                            ow2, cnt2 = move_done[q]
                            scalar.wait_ge(s_move[ow2], cnt2)
                            scalar.dma_start(
                                y_view(q), o_view(q)
                            ).then_inc(s_slot[q % 3], 16)

            @block.vector
            def _(vector):
                vector.wait_ge(s_const, 80)
                for q in range(R * NPAIR):
                    if owner_of(q) != "dve":
                        continue
                    for h in range(2):
                        g = 2 * q + h
                        if h == 0 and q >= 3:
                            # o_sb[q%3] free once DMA(q-3) landed
                            s, cnt = pair_dma[q - 3]
                            vector.wait_ge(s_slot[s], cnt)
                        vector.wait_ge(s_pe, op_done[g])
                        nc.vector.tensor_add(
                            o_sb[q % 3][:, ts(h, 2 * D)],
                            op_ps[g % 3][:],
                            bi_sb[:],
                        ).then_inc(s_dve, 1)

    _PROGRAM_CACHE[key] = nc
    return nc


def _prep_inputs(x, W_ve, b_ve, W_lin, b_lin):
    fts, bank_terms = _build_filter_banks()
    n_uniq = fts.shape[1] // (4 * L)
    W_comb = (W_lin.astype(np.float64) @ W_ve.astype(np.float64)).T  # [C, D]
    b_out = W_lin.astype(np.float64) @ b_ve.astype(np.float64) + b_lin.astype(np.float64)
    # xq[p, k*CP + b*C + c] = x[b, c, k*128 + p]
    xq_all = (
        x.reshape(B, C, NCH, L)
        .transpose(3, 2, 0, 1)           # [p, k, b, c]  (b within full B)
        .reshape(L, NCH, B, C)
    )
    wcr = np.tile(W_comb.astype(np.float32), (BPC, 1))          # [128, D]
    bias2 = np.tile(b_out.astype(np.float32), 2)                 # [2*D]
    common = {
        "fts": fts,
        "wcr": np.ascontiguousarray(wcr),
        "bias": np.ascontiguousarray(
            np.broadcast_to(bias2.astype(np.float32), (128, 2 * D))
        ),
        # row 0 of each 32-partition batch strip is ones, rest zeros, so
        # the K=32 bias matmul adds exactly one copy of b_out
        "ones": np.ascontiguousarray(
            (np.arange(128)[:, None] % C == 0).astype(np.float32)
            * np.ones((1, 128), np.float32)
        ),
        "biasr": np.ascontiguousarray(
            np.broadcast_to(b_out.astype(np.float32), (128, D))
        ),
    }
    in_maps = []
    for cc in range(NCORES):
        xq = xq_all[:, :, cc * BPC:(cc + 1) * BPC, :].reshape(L, NCH * CP)
        in_maps.append({"xq": np.ascontiguousarray(xq), **common})
    return in_maps, n_uniq, bank_terms


_RUNNER_CACHE: dict = {}


def _get_runner(n_uniq: int, bank_terms, repeats: int = 1):
    """Compile the Bass program once per `repeats` and cache the jitted
    PJRT executable so repeated kernel invocations skip re-trace /
    re-compile / NEFF re-load."""
    key = (n_uniq, repeats)
    if key in _RUNNER_CACHE:
        return _RUNNER_CACHE[key]

    import jax
    from jax.experimental.shard_map import shard_map
    from jax.sharding import Mesh, NamedSharding, PartitionSpec

    from concourse import bass2jax, mybir

    bass2jax.install_neuronx_cc_hook()
    nc = _build_program(n_uniq, bank_terms, repeats=repeats)

    partition_name = (
        nc.partition_id_tensor.name if nc.partition_id_tensor else None
    )
    in_names: list[str] = []
    out_names: list[str] = []
    out_avals = []
    for alloc in nc.m.functions[0].allocations:
        if not isinstance(alloc, mybir.MemoryLocationSet):
            continue
        name = alloc.memorylocations[0].name
        if alloc.kind == "ExternalInput":
            if name != partition_name:
                in_names.append(name)
        elif alloc.kind == "ExternalOutput":
            out_names.append(name)
            out_avals.append(
                jax.core.ShapedArray(
                    tuple(alloc.tensor_shape), mybir.dt.np(alloc.dtype)
                )
            )
    n_params = len(in_names)
    n_outs = len(out_names)
    all_in = tuple(in_names) + tuple(out_names)
    if partition_name is not None:
        all_in = all_in + (partition_name,)

    def _body(*args):
        operands = list(args)
        if partition_name is not None:
            operands.append(bass2jax.partition_id_tensor())
        return tuple(
            bass2jax._bass_exec_p.bind(
                *operands,
                out_avals=tuple(out_avals),
                in_names=all_in,
                out_names=tuple(out_names),
                lowering_input_output_aliases=(),
                sim_require_finite=True,
                sim_require_nnan=True,
                nc=nc,
            )
        )

    devices = jax.devices()[:NCORES]
    mesh = Mesh(np.asarray(devices), ("core",))
    sharding = NamedSharding(mesh, PartitionSpec("core"))
    sharded = jax.jit(
        shard_map(
            _body,
            mesh=mesh,
            in_specs=(PartitionSpec("core"),) * (n_params + n_outs),
            out_specs=(PartitionSpec("core"),) * n_outs,
            check_rep=False,
        ),
        donate_argnums=tuple(range(n_params, n_params + n_outs)),
        keep_unused=True,
    )

    import jax.numpy as jnp

    out_shapes = [
        (NCORES * a.shape[0], *a.shape[1:]) for a in out_avals
    ]
    out_dtypes = [a.dtype for a in out_avals]

    zeros_fn = jax.jit(
        lambda: tuple(
            jnp.zeros(s, d) for s, d in zip(out_shapes, out_dtypes)
        ),
        out_shardings=(sharding,) * n_outs,
    )

    runner = {
        "nc": nc,
        "fn": sharded,
        "zeros_fn": zeros_fn,
        "in_names": in_names,
        "out_names": out_names,
        "sharding": sharding,
        "out_shapes": out_shapes,
    }
    _RUNNER_CACHE[key] = runner
    return runner


def _device_inputs(runner, in_maps):
    """Concat per-core input maps along axis 0 and put on the mesh."""
    import jax

    arrs = []
    for name in runner["in_names"]:
        cat = np.concatenate([m[name] for m in in_maps], axis=0)
        arrs.append(jax.device_put(cat, runner["sharding"]))
    return arrs


def _run(in_maps, n_uniq, bank_terms, repeats: int = 1, fetch: bool = True,
         dev_inputs=None):
    import jax

    runner = _get_runner(n_uniq, bank_terms, repeats=repeats)
    if dev_inputs is None:
        dev_inputs = _device_inputs(runner, in_maps)
    zeros = runner["zeros_fn"]()
    outs = runner["fn"](*dev_inputs, *zeros)
    if fetch:
        return {n: np.asarray(o) for n, o in zip(runner["out_names"], outs)}
    jax.block_until_ready(outs)
    return None


def kernel(x, W_ve, b_ve, W_lin, b_lin):
    in_maps, n_uniq, bank_terms = _prep_inputs(x, W_ve, b_ve, W_lin, b_lin)
    res = _run(in_maps, n_uniq, bank_terms)
    # cores stacked along axis 0: core c holds batches [4c, 4c+4)
    return np.ascontiguousarray(res["y"]).astype(np.float32, copy=False)

